# revision 3
# baseline (speedup 1.0000x reference)
"""Trainium2 Bass kernel for nn_Block_36438502540029 (involution CNN block).

Structure per core (data-parallel over batch, 2 images/core):
  conv1 (1x1, 512->128) + ReLU -> padded bf16 buffer, chained per quarter
    with the reduce (1x1, 128->32, replicated 4x across partitions) so the
    span pipeline starts early.  x is staged in DRAM quarter-major so the
    first conv1 matmul only waits on 1/4 of the x DMA.
  involution, per image: for each tap pair, span matmuls produce the
    per-pixel kernel maps in PSUM (row groups alternate 0-63/64-127
    between consecutive pairs so their matmuls overlap in the PE array).
    Tap modes balance the element-wise work across engines:
      'A' ACT drains map to bf16 SBUF, DVE bf16 mul vs shifted padded x1
      'B' DVE mul reads the PSUM map directly (fp32, 1x)
      'G' ACT drain + GPSIMD mul
    Products accumulate into the out2 PSUM via identity matmuls, emitted
    with a small lag behind production to keep the PE dense.
  conv3 (1x1, 128->512) + identity residual (extra identity matmul) + bias;
    image 0's conv3 tiles interleave into image 1's pair loop.

All matmuls bf16; PSUM accumulation fp32.
"""
import numpy as np
import ml_dtypes
from contextlib import ExitStack

import concourse.bass as bass
import concourse.tile as tile
from concourse import bacc, mybir
from concourse import bass_utils

bf16 = mybir.dt.bfloat16
f32 = mybir.dt.float32
AF = mybir.ActivationFunctionType
ALU = mybir.AluOpType
BF = ml_dtypes.bfloat16

N_CORES = 8
B, CIN, H, W = 16, 512, 28, 28
BL = B // N_CORES            # images per core
CMID, CRED, G, GCH = 128, 32, 8, 16
KS, PD = 7, 3                # kernel size, pad
HWPX = H * W                 # 784
NPX = BL * HWPX              # 1568
PW = W + 2 * PD              # 34
PIMG = PW * PW               # 1156
NPAD = BL * PIMG + 8         # slack for strided quarter views
NTAP = KS * KS               # 49
NPAIR = (NTAP + 1) // 2      # 25 (last pair single)

QW = 392                     # pixels per quarter (global quarter = img half)
XQ = 4 * QW                  # x columns per global quarter (4 k-chunks)
NEVEN = (NPAIR + 1) // 2     # 13 even pairs
NODD = NPAIR // 2            # 12 odd pairs

# per-pair involution mode: 'A' ACT-drain + DVE mul, 'B' direct-PSUM DVE
# mul, 'G' ACT-drain + GPSIMD mul.  Chosen to balance ACT/DVE/GPSIMD.
PAIR_MODES = {}
for _p in range(NPAIR):
    PAIR_MODES[_p] = 'A'
for _p in (1, 3, 7, 9, 12, 13, 15, 19, 21, 24):
    PAIR_MODES[_p] = 'B'
for _p in (5, 11, 17, 23):
    PAIR_MODES[_p] = 'G'

ACCUM_LAG = 6                # accum-fifo entries held back (1.5 pairs)

_prog_cache = {}


def _build_program(modes, use_stt=False):
    nc = bacc.Bacc("TRN2", num_devices=N_CORES, debug=False)

    dr = {}
    dr["x"] = nc.dram_tensor("x", [128, 4 * NPX], bf16, kind="ExternalInput")
    dr["w1t"] = nc.dram_tensor("w1t", [128, 512], bf16, kind="ExternalInput")
    dr["wrt"] = nc.dram_tensor("wrt", [128, 128], bf16, kind="ExternalInput")
    dr["wsde"] = nc.dram_tensor("wsde", [64, NEVEN * 128], bf16, kind="ExternalInput")
    dr["wsdo"] = nc.dram_tensor("wsdo", [64, NODD * 128], bf16, kind="ExternalInput")
    dr["w3t"] = nc.dram_tensor("w3t", [128, 512], bf16, kind="ExternalInput")
    dr["ident"] = nc.dram_tensor("ident", [128, 128], bf16, kind="ExternalInput")
    dr["b1"] = nc.dram_tensor("b1", [128, 1], f32, kind="ExternalInput")
    dr["brr"] = nc.dram_tensor("brr", [128, 1], f32, kind="ExternalInput")
    dr["bsd"] = nc.dram_tensor("bsd", [128, NTAP], f32, kind="ExternalInput")
    dr["b3"] = nc.dram_tensor("b3", [128, 4], f32, kind="ExternalInput")
    y = nc.dram_tensor("y", [128, 4 * NPX], bf16, kind="ExternalOutput")

    with tile.TileContext(nc) as tc:
        with ExitStack() as ctx:
            const = ctx.enter_context(tc.tile_pool(name="const", bufs=1))
            sbuf = ctx.enter_context(tc.tile_pool(name="sbuf", bufs=1))
            wsmp = ctx.enter_context(tc.tile_pool(name="wsm", bufs=4))
            prodp = ctx.enter_context(tc.tile_pool(name="prod", bufs=10))
            ystg = ctx.enter_context(tc.tile_pool(name="ystg", bufs=3))
            pso = ctx.enter_context(tc.tile_pool(name="pso", bufs=1, space="PSUM"))
            psB = ctx.enter_context(tc.tile_pool(name="psB", bufs=3, space="PSUM"))

            # ---- loads: x spread over 4 queues, quarter-major so conv1
            # starts after 1/4 of the x bytes; weights on the remaining
            # queues ordered by first use ----
            xsb = sbuf.tile([128, 4 * NPX], bf16, name="xsb")
            nc.sync.dma_start(xsb[:, 0:XQ], dr["x"].ap()[:, 0:XQ])
            w1t_sb = const.tile([128, 512], bf16, name="w1t_sb")
            nc.scalar.dma_start(w1t_sb[:], dr["w1t"].ap())
            wrt_sb = const.tile([128, 128], bf16, name="wrt_sb")
            nc.scalar.dma_start(wrt_sb[:], dr["wrt"].ap())
            b1_sb = const.tile([128, 1], f32, name="b1_sb")
            nc.scalar.dma_start(b1_sb[:], dr["b1"].ap())
            brr_sb = const.tile([128, 1], f32, name="brr_sb")
            nc.scalar.dma_start(brr_sb[:], dr["brr"].ap())
            nc.gpsimd.dma_start(xsb[:, 2 * XQ:3 * XQ], dr["x"].ap()[:, 2 * XQ:3 * XQ])
            nc.sync.dma_start(xsb[:, 3 * XQ:4 * XQ], dr["x"].ap()[:, 3 * XQ:4 * XQ])
            wsd_sb = const.tile([128, NEVEN * 128], bf16, name="wsd_sb")
            nc.gpsimd.dma_start(wsd_sb[0:64, :], dr["wsde"].ap())
            nc.gpsimd.dma_start(wsd_sb[64:128, 0:NODD * 128], dr["wsdo"].ap())
            nc.scalar.dma_start(xsb[:, XQ:2 * XQ], dr["x"].ap()[:, XQ:2 * XQ])
            id_sb = const.tile([128, 128], bf16, name="id_sb")
            nc.sync.dma_start(id_sb[:], dr["ident"].ap())
            b3_sb = const.tile([128, 4], f32, name="b3_sb")
            nc.sync.dma_start(b3_sb[:], dr["b3"].ap())
            w3t_sb = const.tile([128, 512], bf16, name="w3t_sb")
            nc.gpsimd.dma_start(w3t_sb[:], dr["w3t"].ap())
            bsd_sb = const.tile([128, NTAP], f32, name="bsd_sb")
            nc.gpsimd.dma_start(bsd_sb[:], dr["bsd"].ap())

            pad_t = sbuf.tile([128, NPAD], bf16, name="pad_t")
            nc.vector.memset(pad_t[:, 0:PIMG], 0.0)
            nc.vector.memset(pad_t[:, PIMG:NPAD], 0.0)
            pad4 = pad_t[:, 0:BL * PIMG].rearrange(
                "p (b i j) -> p b i j", b=BL, i=PW, j=PW)

            w1rep = sbuf.tile([128, NPX], bf16, name="w1rep")
            out2sb = sbuf.tile([128, NPX], bf16, name="out2sb")

            def convred(q):
                # conv1 quarter: out1 = relu(W1' @ x + b1) into padded buf,
                # immediately chained with the reduce for the same quarter.
                b_, hh = q // 2, q % 2
                cps = psB.tile([128, 1024], f32, tag="bc", name=f"c1ps{q}")
                for k in range(4):
                    nc.tensor.matmul(
                        cps[:, 0:QW],
                        w1t_sb[:, 128 * k:128 * (k + 1)],
                        xsb[:, XQ * q + QW * k:XQ * q + QW * (k + 1)],
                        start=(k == 0), stop=(k == 3),
                    )
                dst = pad4[:, b_:b_ + 1, PD + 14 * hh:PD + 14 * hh + 14, PD:PD + W]
                nc.scalar.activation(
                    dst,
                    cps[:, 0:QW].rearrange("p (a i j) -> p a i j",
                                           a=1, i=14, j=W),
                    AF.Relu, bias=b1_sb[:], scale=1.0,
                )
                rps = psB.tile([128, 1024], f32, tag="bc", name=f"redps{q}")
                nc.tensor.matmul(rps[:, 0:QW], wrt_sb, dst,
                                 start=True, stop=True)
                nc.scalar.activation(
                    w1rep[:, QW * q:QW * (q + 1)], rps[:, 0:QW],
                    AF.Relu, bias=brr_sb[:], scale=1.0,
                )

            def conv3_m(himg, m):
                # y_m[image himg] = W3'_m @ out2 + x_m + b3_m
                c3 = psB.tile([128, 1024], f32, tag="bc", name=f"c3_{m}_{himg}")
                for hh in range(2):
                    off = 512 * hh
                    nc.tensor.matmul(
                        c3[:, off:off + QW], w3t_sb[:, 128 * m:128 * (m + 1)],
                        out2sb[:, HWPX * himg + QW * hh:HWPX * himg + QW * (hh + 1)],
                        start=True, stop=False, skip_group_check=True,
                    )
                    nc.tensor.matmul(
                        c3[:, off:off + QW], id_sb,
                        xsb[:, XQ * (2 * himg + hh) + QW * m:
                             XQ * (2 * himg + hh) + QW * (m + 1)],
                        start=False, stop=True, skip_group_check=True,
                    )
                ysb = ystg.tile([128, HWPX], bf16, tag="y", name=f"y{m}_{himg}")
                nc.scalar.activation(
                    ysb[:].rearrange("p (h n) -> p h n", h=2, n=QW),
                    c3[:].rearrange("p (h n) -> p h n", h=2, n=512)[:, :, 0:QW],
                    AF.Identity, bias=b3_sb[:, m:m + 1], scale=1.0)
                for hh in range(2):
                    nc.sync.dma_start(
                        y.ap()[:, NPX * m + HWPX * himg + QW * hh:
                               NPX * m + HWPX * himg + QW * (hh + 1)],
                        ysb[:, QW * hh:QW * (hh + 1)])

            def pad_shift(t, himg, squeeze_q=None):
                di, dj = t // KS - PD, t % KS - PD
                if squeeze_q is None:
                    return pad4[:, himg:himg + 1,
                                PD + di:PD + di + H, PD + dj:PD + dj + W]
                hh = squeeze_q
                r0 = PD + di + 14 * hh
                off = himg * PIMG + r0 * PW + PD + dj
                return pad_t[:, off:off + 14 * PW].rearrange(
                    "p (i j) -> p i j", i=14, j=PW)[:, :, 0:W]

            def involution(himg, pending):
                o2 = pso.tile([128, 1024], f32, tag="o2", name=f"o2_{himg}")
                accum_fifo = []

                def emit_accum(keep):
                    while len(accum_fifo) > keep:
                        pr, t, off, wd = accum_fifo.pop(0)
                        nc.tensor.matmul(
                            o2[:, off:off + wd], id_sb,
                            pr[:, off:off + wd],
                            start=(t == 0), stop=(t == NTAP - 1),
                            skip_group_check=True,
                        )

                for p in range(NPAIR):
                    if pending and 1 <= p <= len(pending):
                        pending[p - 1]()
                    taps = [t for t in (2 * p, 2 * p + 1) if t < NTAP]
                    ns = len(taps)
                    rg = 64 * (p % 2)
                    blk = 128 * (p // 2)
                    mode = modes[p]
                    prods = [prodp.tile([128, HWPX], bf16, tag="prod",
                                        name=f"prod{himg}_{t}")
                             for t in taps]
                    wm = None
                    if mode in ('A', 'G'):
                        wm = wsmp.tile([128, ns * HWPX], bf16, tag="wm",
                                       name=f"wm{himg}_{p}")
                    for hh in range(2):
                        q = 2 * himg + hh
                        bq = psB.tile([128, 1024], f32, tag="bc",
                                      name=f"bc{himg}_{p}_{hh}")
                        for s, t in enumerate(taps):
                            nc.tensor.matmul(
                                bq[:, 512 * s:512 * s + QW],
                                wsd_sb[rg + 32 * s:rg + 32 * (s + 1),
                                       blk:blk + 128],
                                w1rep[rg + 32 * s:rg + 32 * (s + 1),
                                      QW * q:QW * (q + 1)],
                                start=True, stop=True,
                                tile_position=(rg + 32 * s, 0),
                            )
                        emit_accum(ACCUM_LAG)
                        if mode in ('A', 'G'):
                            nc.scalar.activation(
                                wm[:].rearrange("p (s n) -> p s n",
                                                s=ns, n=HWPX)[
                                    :, :, QW * hh:QW * (hh + 1)],
                                bq[:].rearrange("p (s n) -> p s n",
                                                s=2, n=512)[:, 0:ns, 0:QW],
                                AF.Identity, bias=0.0, scale=1.0,
                            )
                        else:
                            for s, t in enumerate(taps):
                                if use_stt:
                                    nc.vector.scalar_tensor_tensor(
                                        prods[s][:, QW * hh:QW * (hh + 1)]
                                        .rearrange("p (i j) -> p i j",
                                                   i=14, j=W),
                                        bq[:, 512 * s:512 * s + QW].rearrange(
                                            "p (i j) -> p i j", i=14, j=W),
                                        bsd_sb[:, t:t + 1],
                                        pad_shift(t, himg, squeeze_q=hh),
                                        ALU.add, ALU.mult,
                                    )
                                else:
                                    nc.vector.tensor_mul(
                                        prods[s][:, QW * hh:QW * (hh + 1)]
                                        .rearrange("p (i j) -> p i j",
                                                   i=14, j=W),
                                        bq[:, 512 * s:512 * s + QW].rearrange(
                                            "p (i j) -> p i j", i=14, j=W),
                                        pad_shift(t, himg, squeeze_q=hh),
                                    )
                    if mode in ('A', 'G'):
                        eng = nc.vector if mode == 'A' else nc.gpsimd
                        for s, t in enumerate(taps):
                            eng.tensor_mul(
                                prods[s][:].rearrange("p (i j) -> p i j",
                                                      i=H, j=W),
                                wm[:, HWPX * s:HWPX * (s + 1)].rearrange(
                                    "p (i j) -> p i j", i=H, j=W),
                                pad_shift(t, himg),
                            )
                    for s, t in enumerate(taps):
                        accum_fifo.append((prods[s], t, 0, 512))
                        accum_fifo.append((prods[s], t, 512, 272))
                emit_accum(0)

                nc.scalar.activation(
                    out2sb[:, HWPX * himg:HWPX * (himg + 1)], o2[:, 0:HWPX],
                    AF.Identity, bias=0.0, scale=1.0)

            convred(0)
            convred(1)
            involution(0, [lambda: convred(2), lambda: convred(3)])
            involution(1, [lambda m=m: conv3_m(0, m) for m in range(4)])
            for m in range(4):
                conv3_m(1, m)

    nc.compile()
    return nc


def get_program(all_direct=False):
    key = "nc_all_direct" if all_direct else "nc"
    if key not in _prog_cache:
        if all_direct:
            modes = {p: 'B' for p in range(NPAIR)}
            _prog_cache[key] = _build_program(modes, use_stt=True)
        else:
            _prog_cache[key] = _build_program(PAIR_MODES, use_stt=False)
    return _prog_cache[key]


def _host_prep(inputs):
    """Fold scales into weights; build per-core DRAM tensor layouts."""
    x = np.asarray(inputs["x"], np.float32)
    W1 = np.asarray(inputs["W1"], np.float32) * np.asarray(inputs["s1"], np.float32)[:, None]
    Wr = np.asarray(inputs["Wr"], np.float32) * np.asarray(inputs["sr"], np.float32)[:, None]
    Ws = np.asarray(inputs["Ws"], np.float32)
    W3 = np.asarray(inputs["W3"], np.float32) * np.asarray(inputs["s3"], np.float32)[:, None]
    b1 = np.asarray(inputs["b1"], np.float32)
    br = np.asarray(inputs["br"], np.float32)
    bs = np.asarray(inputs["bs"], np.float32)
    b3 = np.asarray(inputs["b3"], np.float32)

    w1t = np.ascontiguousarray(
        W1.T.reshape(4, 128, 128).transpose(1, 0, 2).reshape(128, 512)).astype(BF)
    wrt = np.tile(Wr.T, (1, 4)).astype(BF)
    WsT = Ws.reshape(G, NTAP, CRED)  # [g, t, j]
    wsde = np.zeros((64, NEVEN * 128), np.float32)
    wsdo = np.zeros((64, NODD * 128), np.float32)
    for p in range(NPAIR):
        dst = wsde if p % 2 == 0 else wsdo
        i = p // 2
        for s in range(2):
            t = 2 * p + s
            if t >= NTAP:
                continue
            blk = WsT[:, t, :].T  # [j, g]
            dst[32 * s:32 * s + 32, 128 * i:128 * (i + 1)] = np.repeat(
                blk, GCH, axis=1)
    w3t = W3.T.astype(BF)
    ident = np.eye(128, dtype=np.float32).astype(BF)
    bsd = np.repeat(bs.reshape(G, NTAP), GCH, axis=0)
    bsd = np.ascontiguousarray(bsd).astype(np.float32)

    base = {
        "w1t": w1t, "wrt": wrt,
        "wsde": wsde.astype(BF), "wsdo": wsdo.astype(BF),
        "w3t": w3t, "ident": ident,
        "b1": b1.reshape(128, 1).astype(np.float32),
        "brr": np.tile(br, 4).reshape(128, 1).astype(np.float32),
        "bsd": bsd,
        "b3": np.ascontiguousarray(b3.reshape(4, 128).T).astype(np.float32),
    }
    in_maps = []
    for c in range(N_CORES):
        xs = x[BL * c:BL * (c + 1)]
        # quarter-major: col = (2b+hh)*1568 + k*392 + n
        xc = np.ascontiguousarray(
            xs.reshape(BL, 4, 128, 2, QW).transpose(2, 0, 3, 1, 4)
            .reshape(128, 4 * NPX)).astype(BF)
        m = dict(base)
        m["x"] = xc
        in_maps.append(m)
    return in_maps


def _unshard(results):
    out = np.empty((B, CIN, H, W), np.float32)
    for c in range(N_CORES):
        yc = results[c]["y"].astype(np.float32)
        yv = yc.reshape(128, 4, BL, H, W).transpose(2, 1, 0, 3, 4)
        out[BL * c:BL * (c + 1)] = yv.reshape(BL, CIN, H, W)
    return out


def kernel(**inputs):
    # the fast drained path assumes bs == 0 (true for this problem's
    # setup_inputs); nonzero bs routes every pair through the direct path,
    # which applies bs exactly
    all_direct = bool(np.abs(np.asarray(inputs["bs"])).max() > 0)
    nc = get_program(all_direct)
    in_maps = _host_prep(inputs)
    import os
    trace = bool(os.environ.get("KERNEL_TRACE"))
    kw = {}
    if trace:
        import tempfile
        kw = dict(trace=True, tmpdir=tempfile.mkdtemp(prefix="ktr_"))
        try:
            import ntff_shim  # noqa: F401
        except ImportError:
            pass
    res = bass_utils.run_bass_kernel_spmd(
        nc, in_maps, core_ids=list(range(N_CORES)), **kw)
    if trace and res.exec_time_ns is not None:
        prof = os.environ.get("KERNEL_PROFILE_OUT")
        if prof:
            with open(prof, "w") as f:
                f.write(str(res.exec_time_ns))
        print(f"HW exec time: {res.exec_time_ns} ns")
    return _unshard(res.results)
